# revision 28
# baseline (speedup 1.0000x reference)
"""Causal self-attention (B=4, T=2048, C=1024, 16 heads) on 8 trn2 NeuronCores.

Sharding: batch x head-group hybrid. Core c handles batch c//2 and head
group c%2 (8 of 16 heads). Each core computes the qkv projection for its
head group over its batch's tokens, runs causal attention for its 8
heads, and produces a partial c_proj output (contraction over its 512 of
the 1024 y channels). Host sums the two partials per batch, adds b_proj.

PE contracts over the partition dim, so x is needed transposed (x^T).
x is converted to bf16 on the host and x^T is produced by the DMA
engines' XBAR transpose, leaving PE/DVE/ScalarE untouched:
  x^T [c, tok]        DMA transpose, bf16, direct from DRAM
  Q^T, K^T [j, tok]   = W_qk^T x^T   (j head-major, bf16)
  V' [tok, 65]        = x W_v        (bf16; col 64 = ones so that P@V'
                                      also emits softmax denominators)
  S^T [k_tok, q]      = K^T_tile.T Q^T  both heads of a pair land in one
                        2-bank PSUM tile so a single ScalarE exp covers
                        them.
  P = exp(S^T/8)      bf16; causal diagonal blocks masked by a triu
                        multiply; fully-masked columns never computed.
  O' [65, q]          = V'.T P accumulated over k tiles.
  y [128, tok]        per head pair, bf16. Head B's O' rows are shifted
                        into partitions 64..128 by a SBUF->SBUF DMA
                        (compute engines cannot cross partitions).
  out partial [tok, C] = y_pair.T W_proj_rows accumulated over pairs.

Scheduling notes:
  - All matmul operands are bf16 (fp32 matmuls cost 4 cyc/row, bf16 1).
  - Weight/bias/out-store DMAs are issued from the Pool engine (SWDGE)
    so they never contend with the latency-critical HWDGE queue, which
    carries only the x^T transposes and the per-group tail DMAs.
  - Chunk 0's V' runs ct-incrementally over 4 PSUM accumulators so PE
    starts within ~5us of t=0 while the wqk DMAs are still in flight.
  - Attention loops qc-outer so the output projection of chunk qc can be
    emitted (and run) while attention for qc+1 proceeds.
  - V' for all 8 heads of a token tile is staged through one PSUM tile
    and copied out by a single ScalarE copy (v layout [128, tt, h, 65]),
    amortizing the ~185ns ScalarE access latency.
  - Each group's normalization tail (copies -> SBUF DMAs -> reciprocal)
    runs on DVE/DMA in the background; the PE-touching finish (selector
    broadcast matmul + multiply) is deferred into the next group so
    neither PE nor VectorE stalls behind the chain.
Measured end-to-end relative error vs the fp32 reference: ~4e-3.
"""

from contextlib import ExitStack

import numpy as np
import ml_dtypes

import concourse.bass as bass
import concourse.mybir as mybir
import concourse.tile as tile
from concourse import bacc
from concourse.bass_utils import run_bass_kernel_spmd
from concourse.masks import make_identity

F32 = mybir.dt.float32
BF16 = mybir.dt.bfloat16

T = 2048
C = 1024
NH_LOC = 8          # heads per core
HD = 64
J = NH_LOC * HD     # 512 local q/k/v channels
N_CORES = 8
QC = 4              # q chunks of 512
TOK_TILES = 16      # token tiles of 128
C_TILES = 8         # contraction tiles of 128 over C
PAIRS = 4           # head pairs per core


def build_nc(debug_taps=False):
    nc = bacc.Bacc("TRN2", target_bir_lowering=False, debug=False)
    dbg = {}
    if debug_taps:
        dbg["y"] = nc.dram_tensor("dbg_y", [PAIRS * 128, T], BF16,
                                  kind="ExternalOutput")
        dbg["qt"] = nc.dram_tensor("dbg_qt", [PAIRS * 128, T], BF16,
                                   kind="ExternalOutput")
        dbg["kt"] = nc.dram_tensor("dbg_kt", [PAIRS * 128, T], BF16,
                                   kind="ExternalOutput")
        dbg["v"] = nc.dram_tensor("dbg_v", [128, TOK_TILES * NH_LOC * 65],
                                  BF16, kind="ExternalOutput")

    # Weights arrive host-rearranged partition-major so every weight DMA is
    # 128 fat contiguous descriptors (the SWDGE ring holds only 1024 and
    # HWDGE is a serial ~665ns/instruction resource).
    x_d = nc.dram_tensor("x", [T, C], BF16, kind="ExternalInput")
    wq_d = nc.dram_tensor("wq", [128, 4, C_TILES, 128], BF16,
                          kind="ExternalInput")
    wk_d = nc.dram_tensor("wk", [128, 4, C_TILES, 128], BF16,
                          kind="ExternalInput")
    wv_d = nc.dram_tensor("wv", [128, C_TILES, J], BF16, kind="ExternalInput")
    bqk_d = nc.dram_tensor("bqk", [128, 8], F32, kind="ExternalInput")
    bv_d = nc.dram_tensor("bv", [J], BF16, kind="ExternalInput")
    wp_d = nc.dram_tensor("wp", [128, PAIRS, C], BF16, kind="ExternalInput")
    out_d = nc.dram_tensor("out", [T, C], F32, kind="ExternalOutput")

    with tile.TileContext(nc) as tc, ExitStack() as ctx:
        const = ctx.enter_context(tc.tile_pool(name="const", bufs=1))
        wpool = ctx.enter_context(tc.tile_pool(name="w", bufs=1))
        qkv = ctx.enter_context(tc.tile_pool(name="qkv", bufs=1))
        ypool = ctx.enter_context(tc.tile_pool(name="y", bufs=1))
        wk = ctx.enter_context(tc.tile_pool(name="wk", bufs=1))

        # ---- resident weights (bf16) and x^T ----
        bqk_sb = const.tile([128, 8], F32)
        bv_sb = const.tile([1, J], BF16)
        wv_sb = wpool.tile([128, C_TILES, J], BF16, name="wv")
        wq_sb = wpool.tile([128, 4, C_TILES, 128], BF16, name="wq")
        wk_sb = wpool.tile([128, 4, C_TILES, 128], BF16, name="wk")
        wp_sb = wpool.tile([128, PAIRS, C], BF16, name="wp")
        xT = qkv.tile([128, C_TILES, T], BF16, name="xT")

        # DMA order is startup-critical. Each HWDGE queue only sustains ~2
        # DMAs in flight, so use both queues (SP carries x^T transposes,
        # Activation carries weights) with big transfers: the XBAR maps
        # in [tok, c] -> out[p, ct, t] with c = ct*128 + p directly.
        nc.scalar.dma_start(wv_sb[:, 0:4, :], wv_d[:, 0:4, :])
        nc.sync.dma_start(xT[:, 0:4, 0:512], x_d[0:512, 0:512],
                          transpose=True)
        nc.scalar.dma_start(wv_sb[:, 4:C_TILES, :], wv_d[:, 4:C_TILES, :])
        nc.sync.dma_start(xT[:, 4:8, 0:512], x_d[0:512, 512:1024],
                          transpose=True)
        nc.scalar.dma_start(wq_sb[:, 0:2], wq_d[:, 0:2])
        nc.scalar.dma_start(wq_sb[:, 2:4], wq_d[:, 2:4])
        nc.scalar.dma_start(bqk_sb, bqk_d[:, :])
        nc.scalar.dma_start(bv_sb, bv_d[:].rearrange("(a n) -> a n", a=1))
        nc.scalar.dma_start(wk_sb[:, 0:2], wk_d[:, 0:2])
        nc.scalar.dma_start(wk_sb[:, 2:4], wk_d[:, 2:4])
        nc.sync.dma_start(xT[:, :, 512:1024], x_d[512:1024, :],
                          transpose=True)
        nc.scalar.dma_start(wp_sb, wp_d[:, :, :])
        nc.sync.dma_start(xT[:, :, 1024:T], x_d[1024:T, :],
                          transpose=True)

        # ---- constants ----
        # triu2[p, c, f] = 1 iff f >= p, duplicated over c: masks the causal
        # diagonal 128-block of both heads' P in one tensor_tensor op.
        triu2 = const.tile([128, 2, 128], BF16)
        nc.gpsimd.memset(triu2, 0.0)
        nc.gpsimd.affine_select(
            out=triu2, in_=triu2, compare_op=mybir.AluOpType.is_gt,
            fill=1.0, base=0, pattern=[[0, 2], [-1, 128]],
            channel_multiplier=1)
        ones_row = const.tile([1, 128], BF16)
        nc.vector.memset(ones_row, 1.0)
        # bf16 identity: folds the last chunk's stashed partial projection
        # back into PSUM via the PE (no DVE adds on the critical tail)
        ident_bf = const.tile([128, 128], BF16)
        make_identity(nc, ident_bf)
        # selab[p, f] = 1 iff f in [64p, 64p+64): head selector for the
        # reciprocal broadcast matmul (partition-1 memsets are illegal).
        selab = const.tile([2, 128], F32)
        nc.gpsimd.memset(selab, 1.0)
        nc.gpsimd.affine_select(
            out=selab, in_=selab, compare_op=mybir.AluOpType.is_ge,
            fill=0.0, base=0, pattern=[[1, 128]], channel_multiplier=-64)
        nc.gpsimd.affine_select(
            out=selab, in_=selab, compare_op=mybir.AluOpType.is_ge,
            fill=0.0, base=63, pattern=[[-1, 128]], channel_multiplier=64)
        selab_r = const.tile([2, 128], mybir.dt.float32r)
        nc.vector.tensor_copy(selab_r, selab)

        # ---- persistent activations ----
        qt_sb = [qkv.tile([128, T], BF16, name=f"qt{p}") for p in range(PAIRS)]
        kt_sb = [qkv.tile([128, T], BF16, name=f"kt{p}") for p in range(PAIRS)]
        # v layout [128, tt, head, 65]: col 64 = ones (softmax denominator)
        v_sb = qkv.tile([128, TOK_TILES, NH_LOC, 65], BF16, name="v")
        nc.vector.memset(v_sb[:, :, :, 64:65], 1.0)
        y_sb = [ypool.tile([128, T], BF16, name=f"y{p}") for p in range(PAIRS)]

        # ====== fused pipeline: qkv projection chunks overlap attention ====
        # One PSUM pool for the whole kernel (8 banks):
        #   S    [128,1024] x2  exp-pipeline score tiles            4 banks
        #   O    [65,512]   x2  O' accumulators (o_a, o_b)          2 banks
        #   acc  [128,512]  x1  qkv-projection + c_proj accums      1 bank
        #   bcpo [128,512]  x1  recip broadcasts + c_proj           1 bank
        # Chunk 0 runs before attention and borrows the idle S/O banks for
        # 8 parallel ct-incremental accumulators.
        with tc.tile_pool(name="ps", bufs=1, space="PSUM") as psb:
            pending = []

            def flush_one():
                if pending:
                    pending.pop(0)()

            def flush_pending():
                while pending:
                    pending.pop(0)()

            def wqk_view(ct, jt):
                if jt < 4:
                    return wq_sb[:, jt, ct, :]
                return wk_sb[:, jt - 4, ct, :]

            def qk_finish(jt, qc, pm):
                # late chunks run during ScalarE-heavy attention: use DVE
                dst = qt_sb[jt] if jt < 4 else kt_sb[jt - 4]
                if qc >= 2:
                    nc.vector.tensor_scalar_add(
                        dst[:, qc * 512:(qc + 1) * 512], pm,
                        bqk_sb[:, jt:jt + 1])
                else:
                    nc.scalar.activation(
                        dst[:, qc * 512:(qc + 1) * 512], pm,
                        mybir.ActivationFunctionType.Identity,
                        bias=bqk_sb[:, jt:jt + 1])

            def v_finish(tta, pv):
                nc.tensor.matmul(pv, ones_row, bv_sb, start=False, stop=True)
                src = pv.rearrange("p (h w) -> p h w", h=NH_LOC)
                if tta >= 8:
                    nc.vector.tensor_copy(v_sb[:, tta, :, 0:64], src)
                else:
                    nc.scalar.copy(v_sb[:, tta, :, 0:64], src)

            # ---- chunk 0: ct-incremental over 8 PSUM accumulators ----
            a0 = psb.tile([128, 512], F32, tag="acc", bufs=1, name="a0")
            a1 = psb.tile([128, 512], F32, tag="bcpo", bufs=1, name="a1")
            s0 = psb.tile([128, 1024], F32, tag="S", bufs=2, name="s0")
            s1 = psb.tile([128, 1024], F32, tag="S", bufs=2, name="s1")
            o0 = psb.tile([128, 512], F32, tag="O", bufs=2, name="o0")
            o1 = psb.tile([128, 512], F32, tag="O", bufs=2, name="o1")
            accs = [a0, a1, s0[:, 0:512], s0[:, 512:1024],
                    s1[:, 0:512], s1[:, 512:1024], o0, o1]
            # wave 1: V' tt 0..3 ct-incremental (wv lands first)
            for ct in range(C_TILES):
                for tt in range(4):
                    nc.tensor.matmul(
                        accs[tt], xT[:, ct, tt * 128:(tt + 1) * 128],
                        wv_sb[:, ct, :], start=(ct == 0), stop=False)
            for tt in range(4):
                v_finish(tt, accs[tt])
            # wave 2: all Q (wq lands before wk), then all K
            for i, jt in enumerate([0, 1, 2, 3]):
                pm = accs[4 + i]
                for ct in range(C_TILES):
                    nc.tensor.matmul(
                        pm, wqk_view(ct, jt), xT[:, ct, 0:512],
                        start=(ct == 0), stop=(ct == C_TILES - 1))
                qk_finish(jt, 0, pm)
            for i, jt in enumerate([4, 5, 6, 7]):
                pm = accs[i]
                for ct in range(C_TILES):
                    nc.tensor.matmul(
                        pm, wqk_view(ct, jt), xT[:, ct, 0:512],
                        start=(ct == 0), stop=(ct == C_TILES - 1))
                qk_finish(jt, 0, pm)

            # qkv pieces alternate between the acc and bcpo banks so a
            # piece's accumulation never stalls on the previous piece's
            # PSUM->SBUF read (2-deep software pipeline).
            piece_tag = [0]

            def next_tag():
                piece_tag[0] ^= 1
                return "acc" if piece_tag[0] else "bcpo"

            def a_pieces(qc):
                """Emit-later closures computing Q^T/K^T, V' for chunk qc."""
                pieces = []

                def qk_piece(jt):
                    def run():
                        pm = psb.tile([128, 512], F32, tag=next_tag(),
                                      bufs=1, name="pm")
                        for ct in range(C_TILES):
                            nc.tensor.matmul(
                                pm, wqk_view(ct, jt),
                                xT[:, ct, qc * 512:(qc + 1) * 512],
                                start=(ct == 0), stop=(ct == C_TILES - 1))
                        qk_finish(jt, qc, pm)
                    return run

                def v_piece(tt):
                    def run():
                        tta = qc * 4 + tt
                        pv = psb.tile([128, J], F32, tag=next_tag(),
                                      bufs=1, name="pv")
                        for ct in range(C_TILES):
                            nc.tensor.matmul(
                                pv,
                                xT[:, ct, tta * 128:(tta + 1) * 128],
                                wv_sb[:, ct, :],
                                start=(ct == 0), stop=False)
                        v_finish(tta, pv)
                    return run

                for jt in range(8):
                    pieces.append(qk_piece(jt))
                for tt in range(4):
                    pieces.append(v_piece(tt))
                return pieces

            def emit_tail(p, q0, o_a, o_b, last=False):
                # head A rows land aligned; stage sums + head B rows
                nc.vector.tensor_copy(y_sb[p][0:64, q0:q0 + 512],
                                      o_a[0:64, :])
                stg_b = wk.tile([64, 512], BF16, tag="stgb", bufs=3)
                nc.vector.tensor_copy(stg_b, o_b[0:64, :])
                stg_s = wk.tile([65, 1024], F32, tag="stgs", bufs=3)
                nc.vector.tensor_copy(stg_s[64:65, 0:512], o_a[64:65, :])
                nc.vector.tensor_copy(stg_s[64:65, 512:1024], o_b[64:65, :])
                sums = wk.tile([2, 512], F32, tag="sums", bufs=3)
                # the last group's chain is exposed: split it across both
                # HWDGE queues (ScalarE is done with exp by then)
                sq = nc.scalar if last else nc.sync
                sq.dma_start(sums[0:1, :], stg_s[64:65, 0:512])
                sq.dma_start(sums[1:2, :], stg_s[64:65, 512:1024])
                nc.sync.dma_start(y_sb[p][64:128, q0:q0 + 512], stg_b)
                rec = wk.tile([2, 512], F32, tag="rec", bufs=3)
                nc.vector.reciprocal_approx_fast(rec, sums)
                # f32r matmul inputs must come from a rounding producer
                rec_r = wk.tile([2, 512], mybir.dt.float32r, tag="recr",
                                bufs=2)
                nc.vector.tensor_copy(rec_r, rec)

                def fin():
                    # f32r runs 1 cyc/row vs fp32's 4 (values are exact 0/1
                    # selector rows times fp32 reciprocals; f32r's reduced
                    # multiply precision is irrelevant here)
                    bc = psb.tile([128, 512], F32, tag="bcpo", bufs=1,
                                  name="bc")
                    nc.tensor.matmul(bc, selab_r, rec_r,
                                     start=True, stop=True)
                    nc.vector.tensor_mul(y_sb[p][:, q0:q0 + 512],
                                         y_sb[p][:, q0:q0 + 512], bc)
                pending.append(fin)

            def make_proj_piece(tt, oc, tag="bcpo", bufs=1, on_scalar=False):
                # one output-projection accumulation for token tile tt
                # (needs all 4 pairs' y columns for tt normalized)
                def proj():
                    po = psb.tile([128, 512], F32, tag=tag, bufs=bufs,
                                  name="po")
                    for p in range(PAIRS):
                        nc.tensor.matmul(
                            po,
                            y_sb[p][:, tt * 128:(tt + 1) * 128],
                            wp_sb[:, p, oc * 512:(oc + 1) * 512],
                            start=(p == 0), stop=(p == PAIRS - 1))
                    ob = wk.tile([128, 512], F32, tag="ob", bufs=6)
                    if on_scalar:
                        nc.scalar.copy(ob, po)
                    else:
                        nc.vector.tensor_copy(ob, po)
                    nc.gpsimd.dma_start(
                        out_d[tt * 128:(tt + 1) * 128,
                              oc * 512:(oc + 1) * 512], ob)
                return proj

            proj3_stash = {}

            def make_proj3_part(tt, oc):
                # pairs 0..2 of the last chunk's projection, stashed to SBUF
                # as bf16 so the tail can re-add it through the PE
                def part():
                    po = psb.tile([128, 512], F32, tag=next_tag(), bufs=1,
                                  name="po3p")
                    for p in range(3):
                        nc.tensor.matmul(
                            po,
                            y_sb[p][:, tt * 128:(tt + 1) * 128],
                            wp_sb[:, p, oc * 512:(oc + 1) * 512],
                            start=(p == 0), stop=(p == 2))
                    st = wk.tile([128, 512], BF16, tag="stash", bufs=8)
                    nc.vector.tensor_copy(st, po)
                    proj3_stash[(tt, oc)] = st
                return part

            def make_proj3_fin(tt):
                # pair-3 term + PE-folded stash add, one piece per token
                # tile; copies alternate ScalarE/DVE, stores alternate the
                # two HWDGE queues (each sustains only ~2 DMAs in flight)
                def fin3():
                    po = psb.tile([128, 1024], F32, tag="S", bufs=2,
                                  name="po3f")
                    for oc in range(2):
                        nc.tensor.matmul(
                            po[:, oc * 512:(oc + 1) * 512],
                            ident_bf, proj3_stash[(tt, oc)],
                            start=True, stop=False)
                        nc.tensor.matmul(
                            po[:, oc * 512:(oc + 1) * 512],
                            y_sb[3][:, tt * 128:(tt + 1) * 128],
                            wp_sb[:, 3, oc * 512:(oc + 1) * 512],
                            start=False, stop=True)
                    ob = wk.tile([128, 1024], F32, tag="ob3", bufs=2)
                    if tt % 2:
                        nc.scalar.copy(ob, po)
                        nc.scalar.dma_start(
                            out_d[tt * 128:(tt + 1) * 128, :], ob)
                    else:
                        nc.vector.tensor_copy(ob, po)
                        nc.sync.dma_start(
                            out_d[tt * 128:(tt + 1) * 128, :], ob)
                return fin3

            a_left = [0] * QC  # un-flushed A pieces per chunk

            def count_piece(piece, qc):
                def run():
                    a_left[qc] -= 1
                    piece()
                return run

            for qc in range(QC):
                q0 = qc * 512
                n_kt = 4 * (qc + 1)
                if qc + 1 < QC:
                    pcs = a_pieces(qc + 1)
                    a_left[qc + 1] = len(pcs)
                    pending.extend(count_piece(pc, qc + 1) for pc in pcs)
                # emission barrier: attention for qc depends on chunk qc's
                # Q/K/V writes being *emitted* (Tile tracks deps in trace
                # order); normally a no-op since pieces drain during qc-1.
                while a_left[qc] > 0:
                    flush_one()
                for p in range(PAIRS):
                    o_a = psb.tile([65, 512], F32, tag="O", bufs=2, name="o_a")
                    o_b = psb.tile([65, 512], F32, tag="O", bufs=2, name="o_b")
                    staged = {}

                    def emit_s(kt):
                        off = max(0, kt * 128 - q0)
                        # S for both heads in one 2-bank psum tile so one
                        # ScalarE exp covers both
                        s_ab = psb.tile([128, 1024], F32, tag="S", bufs=2,
                                        name="s_ab")
                        for half in range(2):
                            r0, r1 = half * 64, half * 64 + 64
                            nc.tensor.matmul(
                                s_ab[:, half * 512 + off:half * 512 + 512],
                                kt_sb[p][r0:r1, kt * 128:(kt + 1) * 128],
                                qt_sb[p][r0:r1, q0 + off:q0 + 512],
                                start=True, stop=True)
                        staged[kt] = (s_ab, off)

                    def emit_consume(kt):
                        s_ab, off = staged.pop(kt)
                        p_ab = wk.tile([128, 1024], BF16, tag="P", bufs=8,
                                       name="p_ab")
                        s3 = s_ab.rearrange("p (c w) -> p c w", c=2)
                        p3 = p_ab.rearrange("p (c w) -> p c w", c=2)
                        nc.scalar.activation(
                            p3[:, :, off:512], s3[:, :, off:512],
                            mybir.ActivationFunctionType.Exp, scale=0.125)
                        if kt * 128 >= q0:  # causal diagonal block
                            nc.vector.tensor_mul(
                                p3[:, :, off:off + 128],
                                p3[:, :, off:off + 128], triu2)
                        first, last = (kt == 0), (kt == n_kt - 1)
                        nc.tensor.matmul(o_a[:, off:512],
                                         v_sb[:, kt, 2 * p, :],
                                         p_ab[:, off:512],
                                         start=first, stop=last)
                        nc.tensor.matmul(o_b[:, off:512],
                                         v_sb[:, kt, 2 * p + 1, :],
                                         p_ab[:, 512 + off:1024],
                                         start=first, stop=last)

                    # software pipeline: emit S(kt) one step ahead of its
                    # exp/mask/PV consumers so PE never waits for ScalarE.
                    # Pending PE-only pieces are spread evenly across the
                    # remaining attention windows of this chunk: attention
                    # alone is ScalarE-bound (~1038ns vs ~853ns PE per
                    # k-tile), so the pieces are what keep PE busy.
                    quota = -(-len(pending) // (PAIRS - p))
                    flushed = 0
                    for kt in range(n_kt + 1):
                        if kt < n_kt:
                            emit_s(kt)
                        if kt >= 1:
                            target = quota * kt // n_kt
                            while flushed < target and pending:
                                flush_one()
                                flushed += 1
                            emit_consume(kt - 1)
                    emit_tail(p, q0, o_a, o_b,
                              last=(qc == QC - 1 and p == PAIRS - 1))
                    if qc == QC - 1 and p == 2:
                        # last chunk: the projection would otherwise be an
                        # un-overlapped tail after pair 3 finishes. Run the
                        # pairs-0..2 partial sums during pair 3's attention,
                        # stash them in SBUF, and only the pair-3 term +
                        # add remains at the end.
                        for tt in range(qc * 4, qc * 4 + 4):
                            for oc in range(2):
                                pending.append(make_proj3_part(tt, oc))
                if qc == QC - 1:
                    for tt in range(qc * 4, qc * 4 + 4):
                        pending.append(make_proj3_fin(tt))
                else:
                    for tt in range(qc * 4, qc * 4 + 4):
                        for oc in range(2):
                            pending.append(make_proj_piece(tt, oc))
            flush_pending()
            if debug_taps:
                for p in range(PAIRS):
                    nc.sync.dma_start(dbg["y"][p * 128:(p + 1) * 128, :],
                                      y_sb[p])
                    nc.sync.dma_start(dbg["qt"][p * 128:(p + 1) * 128, :],
                                      qt_sb[p])
                    nc.sync.dma_start(dbg["kt"][p * 128:(p + 1) * 128, :],
                                      kt_sb[p])
                nc.sync.dma_start(
                    dbg["v"], v_sb.rearrange("p a b c -> p (a b c)"))

    nc.compile()
    return nc


_NC_CACHE = {}


def _get_nc():
    if "nc" not in _NC_CACHE:
        _NC_CACHE["nc"] = build_nc()
    return _NC_CACHE["nc"]


def _pmajor(w):
    """[C_TILES*128, F] -> [128, C_TILES, F] partition-major, contiguous."""
    ct = w.shape[0] // 128
    return np.ascontiguousarray(
        w.reshape(ct, 128, w.shape[1]).transpose(1, 0, 2))


def shard_inputs(x, W_attn, b_attn, W_proj):
    """Per-core input maps. Core c: batch c//2, head group c%2."""
    bf = ml_dtypes.bfloat16
    x = np.asarray(x, dtype=np.float32)
    W_attn = np.asarray(W_attn, dtype=np.float32)
    b_attn = np.asarray(b_attn, dtype=np.float32)
    W_proj = np.asarray(W_proj, dtype=np.float32)
    in_maps = []
    for c in range(N_CORES):
        b, hg = c // 2, c % 2
        qs, ks, vs = hg * J, C + hg * J, 2 * C + hg * J
        # wq/wk jt-major: [p, jt, ct, jcol]
        wq = np.ascontiguousarray(
            W_attn[:, qs:qs + J].astype(bf)
            .reshape(8, 128, 4, 128).transpose(1, 2, 0, 3))
        wkk = np.ascontiguousarray(
            W_attn[:, ks:ks + J].astype(bf)
            .reshape(8, 128, 4, 128).transpose(1, 2, 0, 3))
        wv = _pmajor(W_attn[:, vs:vs + J].astype(bf))
        # bqk[p, jt]: bias for channel jt*128 + p (q for jt<4, k for jt>=4)
        bqk = np.ascontiguousarray(
            np.concatenate([b_attn[qs:qs + J], b_attn[ks:ks + J]])
            .reshape(8, 128).T)
        bv = np.ascontiguousarray(b_attn[vs:vs + J]).astype(bf)
        wp = _pmajor(W_proj[hg * J:(hg + 1) * J, :].astype(bf))
        in_maps.append({
            "x": np.ascontiguousarray(x[b]).astype(bf),
            "wq": wq, "wk": wkk, "wv": wv, "bqk": bqk, "bv": bv, "wp": wp,
        })
    return in_maps


def kernel(x, W_attn, b_attn, W_proj, b_proj):
    nc = _get_nc()
    in_maps = shard_inputs(x, W_attn, b_attn, W_proj)
    res = run_bass_kernel_spmd(nc, in_maps, list(range(N_CORES)))
    b_proj = np.asarray(b_proj, dtype=np.float32)
    outs = []
    for b in range(4):
        partial = res.results[2 * b]["out"] + res.results[2 * b + 1]["out"]
        outs.append(partial + b_proj[None, :])
    return np.stack(outs, axis=0)


# revision 41
# speedup vs baseline: 1.0035x; 1.0035x over previous
"""Causal self-attention (B=4, T=2048, C=1024, 16 heads) on 8 trn2 NeuronCores.

Sharding: batch x head-group hybrid. Core c handles batch c//2 and head
group c%2 (8 of 16 heads). Each core computes the qkv projection for its
head group over its batch's tokens, runs causal attention for its 8
heads, and produces a partial c_proj output (contraction over its 512 of
the 1024 y channels). Host sums the two partials per batch, adds b_proj.

PE contracts over the partition dim, so x is needed transposed (x^T).
x is converted to bf16 on the host and x^T is produced by the DMA
engines' XBAR transpose, leaving PE/DVE/ScalarE untouched:
  x^T [c, tok]        DMA transpose, bf16, direct from DRAM
  Q^T, K^T [j, tok]   = W_qk^T x^T   (j head-major, bf16)
  V' [tok, 65]        = x W_v        (bf16; col 64 = ones so that P@V'
                                      also emits softmax denominators)
  S^T [k_tok, q]      = K^T_tile.T Q^T  both heads of a pair land in one
                        2-bank PSUM tile so a single ScalarE exp covers
                        them.
  P = exp(S^T/8)      bf16; causal diagonal blocks masked by a triu
                        multiply; fully-masked columns never computed.
  O' [65, q]          = V'.T P accumulated over k tiles.
  y [128, tok]        per head pair, bf16. Head B's O' rows are shifted
                        into partitions 64..128 by a SBUF->SBUF DMA
                        (compute engines cannot cross partitions).
  out partial [tok, C] = y_pair.T W_proj_rows accumulated over pairs.

Scheduling notes:
  - All matmul operands are bf16 (fp32 matmuls cost 4 cyc/row, bf16 1).
  - Weight/bias/out-store DMAs are issued from the Pool engine (SWDGE)
    so they never contend with the latency-critical HWDGE queue, which
    carries only the x^T transposes and the per-group tail DMAs.
  - Chunk 0's V' runs ct-incrementally over 4 PSUM accumulators so PE
    starts within ~5us of t=0 while the wqk DMAs are still in flight.
  - Attention loops qc-outer so the output projection of chunk qc can be
    emitted (and run) while attention for qc+1 proceeds.
  - V' for all 8 heads of a token tile is staged through one PSUM tile
    and copied out by a single ScalarE copy (v layout [128, tt, h, 65]),
    amortizing the ~185ns ScalarE access latency.
  - Each group's normalization tail (copies -> SBUF DMAs -> reciprocal)
    runs on DVE/DMA in the background; the PE-touching finish (selector
    broadcast matmul + multiply) is deferred into the next group so
    neither PE nor VectorE stalls behind the chain.
Measured end-to-end relative error vs the fp32 reference: ~4e-3.
"""

from contextlib import ExitStack

import numpy as np
import ml_dtypes

import concourse.bass as bass
import concourse.mybir as mybir
import concourse.tile as tile
from concourse import bacc
from concourse.bass_utils import run_bass_kernel_spmd
from concourse.masks import make_identity

F32 = mybir.dt.float32
BF16 = mybir.dt.bfloat16

T = 2048
C = 1024
NH_LOC = 8          # heads per core
HD = 64
J = NH_LOC * HD     # 512 local q/k/v channels
N_CORES = 8
QC = 4              # q chunks of 512
TOK_TILES = 16      # token tiles of 128
C_TILES = 8         # contraction tiles of 128 over C
PAIRS = 4           # head pairs per core


def build_nc(debug_taps=False):
    nc = bacc.Bacc("TRN2", target_bir_lowering=False, debug=False)
    dbg = {}
    if debug_taps:
        dbg["y"] = nc.dram_tensor("dbg_y", [PAIRS * 128, T], BF16,
                                  kind="ExternalOutput")
        dbg["qt"] = nc.dram_tensor("dbg_qt", [PAIRS * 128, T], BF16,
                                   kind="ExternalOutput")
        dbg["kt"] = nc.dram_tensor("dbg_kt", [PAIRS * 128, T], BF16,
                                   kind="ExternalOutput")
        dbg["v"] = nc.dram_tensor("dbg_v", [128, TOK_TILES * NH_LOC * 65],
                                  BF16, kind="ExternalOutput")

    # Weights arrive host-rearranged partition-major so every weight DMA is
    # 128 fat contiguous descriptors (the SWDGE ring holds only 1024 and
    # HWDGE is a serial ~665ns/instruction resource).
    x_d = nc.dram_tensor("x", [T, C], BF16, kind="ExternalInput")
    wq_d = nc.dram_tensor("wq", [128, 4, C_TILES, 128], BF16,
                          kind="ExternalInput")
    wk_d = nc.dram_tensor("wk", [128, 4, C_TILES, 128], BF16,
                          kind="ExternalInput")
    wv_d = nc.dram_tensor("wv", [128, C_TILES, J], BF16, kind="ExternalInput")
    bqk_d = nc.dram_tensor("bqk", [128, 8], F32, kind="ExternalInput")
    bv_d = nc.dram_tensor("bv", [J], BF16, kind="ExternalInput")
    wp_d = nc.dram_tensor("wp", [128, PAIRS, C], BF16, kind="ExternalInput")
    out_d = nc.dram_tensor("out", [T, C], F32, kind="ExternalOutput")

    with tile.TileContext(nc) as tc, ExitStack() as ctx:
        const = ctx.enter_context(tc.tile_pool(name="const", bufs=1))
        wpool = ctx.enter_context(tc.tile_pool(name="w", bufs=1))
        qkv = ctx.enter_context(tc.tile_pool(name="qkv", bufs=1))
        ypool = ctx.enter_context(tc.tile_pool(name="y", bufs=1))
        wk = ctx.enter_context(tc.tile_pool(name="wk", bufs=1))

        # ---- resident weights (bf16) and x^T ----
        bqk_sb = const.tile([128, 8], F32)
        bv_sb = const.tile([1, J], BF16)
        wv_sb = wpool.tile([128, C_TILES, J], BF16, name="wv")
        wq_sb = wpool.tile([128, 4, C_TILES, 128], BF16, name="wq")
        wk_sb = wpool.tile([128, 4, C_TILES, 128], BF16, name="wk")
        wp_sb = wpool.tile([128, PAIRS, C], BF16, name="wp")
        xT = qkv.tile([128, C_TILES, T], BF16, name="xT")

        # DMA order is startup-critical. Each HWDGE queue only sustains ~2
        # DMAs in flight, so use both queues (SP carries x^T transposes,
        # Activation carries weights) with big transfers: the XBAR maps
        # in [tok, c] -> out[p, ct, t] with c = ct*128 + p directly.
        nc.scalar.dma_start(wv_sb[:, 0:4, :], wv_d[:, 0:4, :])
        nc.sync.dma_start(xT[:, 0:4, 0:512], x_d[0:512, 0:512],
                          transpose=True)
        nc.scalar.dma_start(wv_sb[:, 4:C_TILES, :], wv_d[:, 4:C_TILES, :])
        nc.sync.dma_start(xT[:, 4:8, 0:512], x_d[0:512, 512:1024],
                          transpose=True)
        nc.scalar.dma_start(wq_sb[:, 0:2], wq_d[:, 0:2])
        nc.scalar.dma_start(wq_sb[:, 2:4], wq_d[:, 2:4])
        nc.scalar.dma_start(bqk_sb, bqk_d[:, :])
        nc.scalar.dma_start(bv_sb, bv_d[:].rearrange("(a n) -> a n", a=1))
        nc.sync.dma_start(xT[:, :, 512:1024], x_d[512:1024, :],
                          transpose=True)
        # wk rides the sync queue behind chunk-1 x^T so the (cheap) wp and
        # chunk 2-3 x^T transfers can never jump ahead of it on the DMA bus
        nc.sync.dma_start(wk_sb[:, 0:2], wk_d[:, 0:2])
        nc.sync.dma_start(wk_sb[:, 2:4], wk_d[:, 2:4])
        nc.scalar.dma_start(wp_sb, wp_d[:, :, :])
        nc.sync.dma_start(xT[:, :, 1024:1536], x_d[1024:1536, :],
                          transpose=True)
        nc.sync.dma_start(xT[:, :, 1536:T], x_d[1536:T, :],
                          transpose=True)

        # ---- constants ----
        # triu2[p, c, f] = 1 iff f >= p, duplicated over c: masks the causal
        # diagonal 128-block of both heads' P in one tensor_tensor op.
        triu2 = const.tile([128, 2, 128], BF16)
        nc.gpsimd.memset(triu2, 0.0)
        nc.gpsimd.affine_select(
            out=triu2, in_=triu2, compare_op=mybir.AluOpType.is_gt,
            fill=1.0, base=0, pattern=[[0, 2], [-1, 128]],
            channel_multiplier=1)
        ones_row = const.tile([1, 128], BF16)
        nc.vector.memset(ones_row, 1.0)
        # bf16 identity: folds the last chunk's stashed partial projection
        # back into PSUM via the PE (no DVE adds on the critical tail)
        ident_bf = const.tile([128, 128], BF16)
        make_identity(nc, ident_bf)
        # selab[p, f] = 1 iff f in [64p, 64p+64): head selector for the
        # reciprocal broadcast matmul (partition-1 memsets are illegal).
        selab = const.tile([2, 128], F32)
        nc.gpsimd.memset(selab, 1.0)
        nc.gpsimd.affine_select(
            out=selab, in_=selab, compare_op=mybir.AluOpType.is_ge,
            fill=0.0, base=0, pattern=[[1, 128]], channel_multiplier=-64)
        nc.gpsimd.affine_select(
            out=selab, in_=selab, compare_op=mybir.AluOpType.is_ge,
            fill=0.0, base=63, pattern=[[-1, 128]], channel_multiplier=64)
        selab_r = const.tile([2, 128], mybir.dt.float32r)
        nc.vector.tensor_copy(selab_r, selab)
        # e65[p, f] = 1 iff p == 64: broadcasts the denominator row of the
        # last group's O' accumulator to all 128 partitions via the PE,
        # skipping the cross-partition sums DMA on the exposed tail chain.
        # Unused rows are zeroed so 0 * garbage never makes a NaN.
        e65 = const.tile([128, 128], F32)
        nc.gpsimd.memset(e65, 1.0)
        nc.gpsimd.affine_select(
            out=e65, in_=e65, compare_op=mybir.AluOpType.is_ge,
            fill=0.0, base=-64, pattern=[[0, 128]], channel_multiplier=1)
        nc.gpsimd.affine_select(
            out=e65, in_=e65, compare_op=mybir.AluOpType.is_ge,
            fill=0.0, base=64, pattern=[[0, 128]], channel_multiplier=-1)
        e65_r = const.tile([128, 128], mybir.dt.float32r)
        nc.vector.tensor_copy(e65_r, e65)
        rec65 = const.tile([128, 1024], F32)
        nc.vector.memset(rec65, 0.0)
        recr65 = const.tile([128, 1024], mybir.dt.float32r)
        nc.vector.tensor_copy(recr65, rec65)

        # ---- persistent activations ----
        qt_sb = [qkv.tile([128, T], BF16, name=f"qt{p}") for p in range(PAIRS)]
        kt_sb = [qkv.tile([128, T], BF16, name=f"kt{p}") for p in range(PAIRS)]
        # v layout [128, tt, head, 65]: col 64 = ones (softmax denominator)
        v_sb = qkv.tile([128, TOK_TILES, NH_LOC, 65], BF16, name="v")
        nc.vector.memset(v_sb[:, :, :, 64:65], 1.0)
        y_sb = [ypool.tile([128, T], BF16, name=f"y{p}") for p in range(PAIRS)]

        # ====== fused pipeline: qkv projection chunks overlap attention ====
        # One PSUM pool for the whole kernel (8 banks):
        #   S    [128,1024] x2  exp-pipeline score tiles            4 banks
        #   O    [65,512]   x2  O' accumulators (o_a, o_b)          2 banks
        #   acc  [128,512]  x1  qkv-projection + c_proj accums      1 bank
        #   bcpo [128,512]  x1  recip broadcasts + c_proj           1 bank
        # Chunk 0 runs before attention and borrows the idle S/O banks for
        # 8 parallel ct-incremental accumulators.
        with tc.tile_pool(name="ps", bufs=1, space="PSUM") as psb:
            pending = []

            def flush_one():
                if pending:
                    pending.pop(0)()

            def flush_pending():
                while pending:
                    pending.pop(0)()

            def wqk_view(ct, jt):
                if jt < 4:
                    return wq_sb[:, jt, ct, :]
                return wk_sb[:, jt - 4, ct, :]

            def qk_finish(jt, qc, pm):
                # late chunks run during ScalarE-heavy attention: use DVE
                dst = qt_sb[jt] if jt < 4 else kt_sb[jt - 4]
                if qc >= 2:
                    nc.vector.tensor_scalar_add(
                        dst[:, qc * 512:(qc + 1) * 512], pm,
                        bqk_sb[:, jt:jt + 1])
                else:
                    nc.scalar.activation(
                        dst[:, qc * 512:(qc + 1) * 512], pm,
                        mybir.ActivationFunctionType.Identity,
                        bias=bqk_sb[:, jt:jt + 1])

            def v_finish(tta, pv):
                nc.tensor.matmul(pv, ones_row, bv_sb, start=False, stop=True)
                src = pv.rearrange("p (h w) -> p h w", h=NH_LOC)
                if tta >= 8:
                    nc.vector.tensor_copy(v_sb[:, tta, :, 0:64], src)
                else:
                    nc.scalar.copy(v_sb[:, tta, :, 0:64], src)

            # ---- chunk 0: ct-incremental over 8 PSUM accumulators ----
            a0 = psb.tile([128, 512], F32, tag="acc", bufs=1, name="a0")
            a1 = psb.tile([128, 512], F32, tag="bcpo", bufs=1, name="a1")
            s0 = psb.tile([128, 1024], F32, tag="S", bufs=2, name="s0")
            s1 = psb.tile([128, 1024], F32, tag="S", bufs=2, name="s1")
            o0 = psb.tile([128, 512], F32, tag="O", bufs=2, name="o0")
            o1 = psb.tile([128, 512], F32, tag="O", bufs=2, name="o1")
            accs = [a0, a1, s0[:, 0:512], s0[:, 512:1024],
                    s1[:, 0:512], s1[:, 512:1024], o0, o1]
            # wave 1: V' tt 0..3 ct-incremental (wv lands first)
            for ct in range(C_TILES):
                for tt in range(4):
                    nc.tensor.matmul(
                        accs[tt], xT[:, ct, tt * 128:(tt + 1) * 128],
                        wv_sb[:, ct, :], start=(ct == 0), stop=False)
            for tt in range(4):
                v_finish(tt, accs[tt])
            # wave 2: all Q (wq lands before wk), then all K
            for i, jt in enumerate([0, 1, 2, 3]):
                pm = accs[4 + i]
                for ct in range(C_TILES):
                    nc.tensor.matmul(
                        pm, wqk_view(ct, jt), xT[:, ct, 0:512],
                        start=(ct == 0), stop=(ct == C_TILES - 1))
                qk_finish(jt, 0, pm)
            for i, jt in enumerate([4, 5, 6, 7]):
                pm = accs[i]
                for ct in range(C_TILES):
                    nc.tensor.matmul(
                        pm, wqk_view(ct, jt), xT[:, ct, 0:512],
                        start=(ct == 0), stop=(ct == C_TILES - 1))
                qk_finish(jt, 0, pm)

            # qkv pieces alternate between the acc and bcpo banks so a
            # piece's accumulation never stalls on the previous piece's
            # PSUM->SBUF read (2-deep software pipeline).
            piece_tag = [0]

            def next_tag():
                piece_tag[0] ^= 1
                return "acc" if piece_tag[0] else "bcpo"

            def a_pieces(qc):
                """Emit-later closures computing Q^T/K^T, V' for chunk qc."""
                pieces = []

                def qk_piece(jt):
                    def run():
                        pm = psb.tile([128, 512], F32, tag=next_tag(),
                                      bufs=1, name="pm")
                        for ct in range(C_TILES):
                            nc.tensor.matmul(
                                pm, wqk_view(ct, jt),
                                xT[:, ct, qc * 512:(qc + 1) * 512],
                                start=(ct == 0), stop=(ct == C_TILES - 1))
                        qk_finish(jt, qc, pm)
                    return run

                def v_piece(tt):
                    def run():
                        tta = qc * 4 + tt
                        pv = psb.tile([128, J], F32, tag=next_tag(),
                                      bufs=1, name="pv")
                        for ct in range(C_TILES):
                            nc.tensor.matmul(
                                pv,
                                xT[:, ct, tta * 128:(tta + 1) * 128],
                                wv_sb[:, ct, :],
                                start=(ct == 0), stop=False)
                        v_finish(tta, pv)
                    return run

                for jt in range(8):
                    pieces.append(qk_piece(jt))
                for tt in range(4):
                    pieces.append(v_piece(tt))
                return pieces

            def emit_tail(p, q0, o_a, o_b, last=False):
                # head A rows land aligned; stage sums + head B rows
                nc.vector.tensor_copy(y_sb[p][0:64, q0:q0 + 512],
                                      o_a[0:64, :])
                stg_b = wk.tile([64, 512], BF16, tag="stgb", bufs=3)
                nc.vector.tensor_copy(stg_b, o_b[0:64, :])
                stg_s = wk.tile([65, 1024], F32, tag="stgs", bufs=2)
                nc.vector.tensor_copy(stg_s[64:65, 0:512], o_a[64:65, :])
                nc.vector.tensor_copy(stg_s[64:65, 512:1024], o_b[64:65, :])
                nc.sync.dma_start(y_sb[p][64:128, q0:q0 + 512], stg_b)
                if last:
                    # exposed tail: broadcast the denominator row via the
                    # PE instead of the ~3us cross-partition DMA round trip
                    nc.vector.reciprocal_approx_fast(
                        rec65[64:65, :], stg_s[64:65, :])
                    nc.vector.tensor_copy(recr65[64:65, :],
                                          rec65[64:65, :])

                    def fin():
                        # bc2[p, :] = (rec_a | rec_b) for every p: head A
                        # rows read the left half, head B rows the right
                        bc2 = psb.tile([128, 1024], F32, tag="S", bufs=2,
                                       name="bc2")
                        nc.tensor.matmul(bc2[:, 0:512], e65_r,
                                         recr65[:, 0:512],
                                         start=True, stop=True)
                        nc.tensor.matmul(bc2[:, 512:1024], e65_r,
                                         recr65[:, 512:1024],
                                         start=True, stop=True)
                        nc.vector.tensor_mul(y_sb[p][0:64, q0:q0 + 512],
                                             y_sb[p][0:64, q0:q0 + 512],
                                             bc2[0:64, 0:512])
                        nc.vector.tensor_mul(y_sb[p][64:128, q0:q0 + 512],
                                             y_sb[p][64:128, q0:q0 + 512],
                                             bc2[64:128, 512:1024])
                    pending.append(fin)
                    return
                sums = wk.tile([2, 512], F32, tag="sums", bufs=3)
                nc.sync.dma_start(sums[0:1, :], stg_s[64:65, 0:512])
                nc.sync.dma_start(sums[1:2, :], stg_s[64:65, 512:1024])
                rec = wk.tile([2, 512], F32, tag="rec", bufs=3)
                nc.vector.reciprocal_approx_fast(rec, sums)
                # f32r matmul inputs must come from a rounding producer
                rec_r = wk.tile([2, 512], mybir.dt.float32r, tag="recr",
                                bufs=2)
                nc.vector.tensor_copy(rec_r, rec)

                def fin():
                    # f32r runs 1 cyc/row vs fp32's 4 (values are exact 0/1
                    # selector rows times fp32 reciprocals; f32r's reduced
                    # multiply precision is irrelevant here)
                    bc = psb.tile([128, 512], F32, tag="bcpo", bufs=1,
                                  name="bc")
                    nc.tensor.matmul(bc, selab_r, rec_r,
                                     start=True, stop=True)
                    nc.vector.tensor_mul(y_sb[p][:, q0:q0 + 512],
                                         y_sb[p][:, q0:q0 + 512], bc)
                fin.is_norm_fin = True
                pending.append(fin)

            def make_proj_piece(tt, oc, tag="bcpo", bufs=1, on_scalar=False):
                # one output-projection accumulation for token tile tt
                # (needs all 4 pairs' y columns for tt normalized)
                def proj():
                    po = psb.tile([128, 512], F32, tag=tag, bufs=bufs,
                                  name="po")
                    for p in range(PAIRS):
                        nc.tensor.matmul(
                            po,
                            y_sb[p][:, tt * 128:(tt + 1) * 128],
                            wp_sb[:, p, oc * 512:(oc + 1) * 512],
                            start=(p == 0), stop=(p == PAIRS - 1))
                    ob = wk.tile([128, 512], F32, tag="ob", bufs=4)
                    if on_scalar:
                        nc.scalar.copy(ob, po)
                    else:
                        nc.vector.tensor_copy(ob, po)
                    nc.gpsimd.dma_start(
                        out_d[tt * 128:(tt + 1) * 128,
                              oc * 512:(oc + 1) * 512], ob)
                return proj

            proj3_stash = {}

            def make_proj3_part(tt, oc):
                # pairs 0..2 of the last chunk's projection, stashed to SBUF
                # as bf16 so the tail can re-add it through the PE
                def part():
                    po = psb.tile([128, 512], F32, tag=next_tag(), bufs=1,
                                  name="po3p")
                    for p in range(3):
                        nc.tensor.matmul(
                            po,
                            y_sb[p][:, tt * 128:(tt + 1) * 128],
                            wp_sb[:, p, oc * 512:(oc + 1) * 512],
                            start=(p == 0), stop=(p == 2))
                    st = wk.tile([128, 512], BF16, tag="stash", bufs=8)
                    nc.vector.tensor_copy(st, po)
                    proj3_stash[(tt, oc)] = st
                return part

            def make_proj3_fin(tt):
                # pair-3 term + PE-folded stash add, one piece per token
                # tile; copies alternate ScalarE/DVE, stores alternate the
                # two HWDGE queues (each sustains only ~2 DMAs in flight)
                def fin3():
                    po = psb.tile([128, 1024], F32, tag="S", bufs=2,
                                  name="po3f")
                    for oc in range(2):
                        nc.tensor.matmul(
                            po[:, oc * 512:(oc + 1) * 512],
                            ident_bf, proj3_stash[(tt, oc)],
                            start=True, stop=False)
                        nc.tensor.matmul(
                            po[:, oc * 512:(oc + 1) * 512],
                            y_sb[3][:, tt * 128:(tt + 1) * 128],
                            wp_sb[:, 3, oc * 512:(oc + 1) * 512],
                            start=False, stop=True)
                    ob = wk.tile([128, 1024], F32, tag="ob3", bufs=3)
                    if tt % 2:
                        nc.scalar.copy(ob, po)
                        nc.scalar.dma_start(
                            out_d[tt * 128:(tt + 1) * 128, :], ob)
                    else:
                        nc.vector.tensor_copy(ob, po)
                        nc.sync.dma_start(
                            out_d[tt * 128:(tt + 1) * 128, :], ob)
                return fin3

            a_left = [0] * QC  # un-flushed A pieces per chunk

            def count_piece(piece, qc):
                def run():
                    a_left[qc] -= 1
                    piece()
                return run

            for qc in range(QC):
                q0 = qc * 512
                n_kt = 4 * (qc + 1)
                if qc + 1 < QC:
                    pcs = a_pieces(qc + 1)
                    a_left[qc + 1] = len(pcs)
                    pending.extend(count_piece(pc, qc + 1) for pc in pcs)
                # emission barrier: attention for qc depends on chunk qc's
                # Q/K/V writes being *emitted* (Tile tracks deps in trace
                # order); normally a no-op since pieces drain during qc-1.
                while a_left[qc] > 0:
                    flush_one()
                for p in range(PAIRS):
                    o_a = psb.tile([65, 512], F32, tag="O", bufs=2, name="o_a")
                    o_b = psb.tile([65, 512], F32, tag="O", bufs=2, name="o_b")
                    staged = {}

                    def emit_s(kt):
                        off = max(0, kt * 128 - q0)
                        # S for both heads in one 2-bank psum tile so one
                        # ScalarE exp covers both
                        s_ab = psb.tile([128, 1024], F32, tag="S", bufs=2,
                                        name="s_ab")
                        for half in range(2):
                            r0, r1 = half * 64, half * 64 + 64
                            nc.tensor.matmul(
                                s_ab[:, half * 512 + off:half * 512 + 512],
                                kt_sb[p][r0:r1, kt * 128:(kt + 1) * 128],
                                qt_sb[p][r0:r1, q0 + off:q0 + 512],
                                start=True, stop=True)
                        staged[kt] = (s_ab, off)

                    def emit_consume(kt):
                        s_ab, off = staged.pop(kt)
                        p_ab = wk.tile([128, 1024], BF16, tag="P", bufs=6,
                                       name="p_ab")
                        s3 = s_ab.rearrange("p (c w) -> p c w", c=2)
                        p3 = p_ab.rearrange("p (c w) -> p c w", c=2)
                        nc.scalar.activation(
                            p3[:, :, off:512], s3[:, :, off:512],
                            mybir.ActivationFunctionType.Exp, scale=0.125)
                        if kt * 128 >= q0:  # causal diagonal block
                            nc.vector.tensor_mul(
                                p3[:, :, off:off + 128],
                                p3[:, :, off:off + 128], triu2)
                        first, last = (kt == 0), (kt == n_kt - 1)
                        nc.tensor.matmul(o_a[:, off:512],
                                         v_sb[:, kt, 2 * p, :],
                                         p_ab[:, off:512],
                                         start=first, stop=last)
                        nc.tensor.matmul(o_b[:, off:512],
                                         v_sb[:, kt, 2 * p + 1, :],
                                         p_ab[:, 512 + off:1024],
                                         start=first, stop=last)

                    # software pipeline: emit S(kt) one step ahead of its
                    # exp/mask/PV consumers so PE never waits for ScalarE.
                    # Pending PE-only pieces are spread evenly across the
                    # remaining attention windows of this chunk: attention
                    # alone is ScalarE-bound (~1038ns vs ~853ns PE per
                    # k-tile), so the pieces are what keep PE busy.
                    quota = -(-len(pending) // (PAIRS - p))
                    flushed = 0
                    for kt in range(n_kt + 1):
                        if kt < n_kt:
                            emit_s(kt)
                        if kt >= 1:
                            target = quota * kt // n_kt
                            while flushed < target and pending:
                                # hold a normalization finish back ~4 k-tiles
                                # so PE never waits on its sums-DMA round
                                # trip (~3.5us)
                                if (kt < 4 and
                                        getattr(pending[0], "is_norm_fin",
                                                False)):
                                    break
                                flush_one()
                                flushed += 1
                            emit_consume(kt - 1)
                    emit_tail(p, q0, o_a, o_b, last=False)
                    if qc == QC - 1 and p == 2:
                        # last chunk: the projection would otherwise be an
                        # un-overlapped tail after pair 3 finishes. Run the
                        # pairs-0..2 partial sums during pair 3's attention,
                        # stash them in SBUF, and only the pair-3 term +
                        # add remains at the end.
                        for tt in range(qc * 4, qc * 4 + 4):
                            for oc in range(2):
                                pending.append(make_proj3_part(tt, oc))
                if qc == QC - 1:
                    for tt in range(qc * 4, qc * 4 + 4):
                        pending.append(make_proj3_fin(tt))
                else:
                    for tt in range(qc * 4, qc * 4 + 4):
                        for oc in range(2):
                            pending.append(make_proj_piece(tt, oc))
            flush_pending()
            if debug_taps:
                for p in range(PAIRS):
                    nc.sync.dma_start(dbg["y"][p * 128:(p + 1) * 128, :],
                                      y_sb[p])
                    nc.sync.dma_start(dbg["qt"][p * 128:(p + 1) * 128, :],
                                      qt_sb[p])
                    nc.sync.dma_start(dbg["kt"][p * 128:(p + 1) * 128, :],
                                      kt_sb[p])
                nc.sync.dma_start(
                    dbg["v"], v_sb.rearrange("p a b c -> p (a b c)"))

    nc.compile()
    return nc


_NC_CACHE = {}


def _get_nc():
    if "nc" not in _NC_CACHE:
        _NC_CACHE["nc"] = build_nc()
    return _NC_CACHE["nc"]


def _pmajor(w):
    """[C_TILES*128, F] -> [128, C_TILES, F] partition-major, contiguous."""
    ct = w.shape[0] // 128
    return np.ascontiguousarray(
        w.reshape(ct, 128, w.shape[1]).transpose(1, 0, 2))


def shard_inputs(x, W_attn, b_attn, W_proj):
    """Per-core input maps. Core c: batch c//2, head group c%2."""
    bf = ml_dtypes.bfloat16
    x = np.asarray(x, dtype=np.float32)
    W_attn = np.asarray(W_attn, dtype=np.float32)
    b_attn = np.asarray(b_attn, dtype=np.float32)
    W_proj = np.asarray(W_proj, dtype=np.float32)
    in_maps = []
    for c in range(N_CORES):
        b, hg = c // 2, c % 2
        qs, ks, vs = hg * J, C + hg * J, 2 * C + hg * J
        # wq/wk jt-major: [p, jt, ct, jcol]
        wq = np.ascontiguousarray(
            W_attn[:, qs:qs + J].astype(bf)
            .reshape(8, 128, 4, 128).transpose(1, 2, 0, 3))
        wkk = np.ascontiguousarray(
            W_attn[:, ks:ks + J].astype(bf)
            .reshape(8, 128, 4, 128).transpose(1, 2, 0, 3))
        wv = _pmajor(W_attn[:, vs:vs + J].astype(bf))
        # bqk[p, jt]: bias for channel jt*128 + p (q for jt<4, k for jt>=4)
        bqk = np.ascontiguousarray(
            np.concatenate([b_attn[qs:qs + J], b_attn[ks:ks + J]])
            .reshape(8, 128).T)
        bv = np.ascontiguousarray(b_attn[vs:vs + J]).astype(bf)
        wp = _pmajor(W_proj[hg * J:(hg + 1) * J, :].astype(bf))
        in_maps.append({
            "x": np.ascontiguousarray(x[b]).astype(bf),
            "wq": wq, "wk": wkk, "wv": wv, "bqk": bqk, "bv": bv, "wp": wp,
        })
    return in_maps


def kernel(x, W_attn, b_attn, W_proj, b_proj):
    nc = _get_nc()
    in_maps = shard_inputs(x, W_attn, b_attn, W_proj)
    res = run_bass_kernel_spmd(nc, in_maps, list(range(N_CORES)))
    b_proj = np.asarray(b_proj, dtype=np.float32)
    outs = []
    for b in range(4):
        partial = res.results[2 * b]["out"] + res.results[2 * b + 1]["out"]
        outs.append(partial + b_proj[None, :])
    return np.stack(outs, axis=0)


# revision 45
# speedup vs baseline: 1.0044x; 1.0009x over previous
"""Causal self-attention (B=4, T=2048, C=1024, 16 heads) on 8 trn2 NeuronCores.

Sharding: batch x head-group hybrid. Core c handles batch c//2 and head
group c%2 (8 of 16 heads). Each core computes the qkv projection for its
head group over its batch's tokens, runs causal attention for its 8
heads, and produces a partial c_proj output (contraction over its 512 of
the 1024 y channels). Host sums the two partials per batch, adds b_proj.

PE contracts over the partition dim, so x is needed transposed (x^T).
x is converted to bf16 on the host and x^T is produced by the DMA
engines' XBAR transpose, leaving PE/DVE/ScalarE untouched:
  x^T [c, tok]        DMA transpose, bf16, direct from DRAM
  Q^T, K^T [j, tok]   = W_qk^T x^T   (j head-major, bf16)
  V' [tok, 65]        = x W_v        (bf16; col 64 = ones so that P@V'
                                      also emits softmax denominators)
  S^T [k_tok, q]      = K^T_tile.T Q^T  both heads of a pair land in one
                        2-bank PSUM tile so a single ScalarE exp covers
                        them.
  P = exp(S^T/8)      bf16; causal diagonal blocks masked by a triu
                        multiply; fully-masked columns never computed.
  O' [65, q]          = V'.T P accumulated over k tiles.
  y [128, tok]        per head pair, bf16. Head B's O' rows are shifted
                        into partitions 64..128 by a SBUF->SBUF DMA
                        (compute engines cannot cross partitions).
  out partial [tok, C] = y_pair.T W_proj_rows accumulated over pairs.

Scheduling notes:
  - All matmul operands are bf16 (fp32 matmuls cost 4 cyc/row, bf16 1).
  - Weight/bias/out-store DMAs are issued from the Pool engine (SWDGE)
    so they never contend with the latency-critical HWDGE queue, which
    carries only the x^T transposes and the per-group tail DMAs.
  - Chunk 0's V' runs ct-incrementally over 4 PSUM accumulators so PE
    starts within ~5us of t=0 while the wqk DMAs are still in flight.
  - Attention loops qc-outer so the output projection of chunk qc can be
    emitted (and run) while attention for qc+1 proceeds.
  - V' for all 8 heads of a token tile is staged through one PSUM tile
    and copied out by a single ScalarE copy (v layout [128, tt, h, 65]),
    amortizing the ~185ns ScalarE access latency.
  - Each group's normalization tail (copies -> SBUF DMAs -> reciprocal)
    runs on DVE/DMA in the background; the PE-touching finish (selector
    broadcast matmul + multiply) is deferred into the next group so
    neither PE nor VectorE stalls behind the chain.
Measured end-to-end relative error vs the fp32 reference: ~4e-3.
"""

from contextlib import ExitStack

import numpy as np
import ml_dtypes

import concourse.bass as bass
import concourse.mybir as mybir
import concourse.tile as tile
from concourse import bacc
from concourse.bass_utils import run_bass_kernel_spmd
from concourse.masks import make_identity

F32 = mybir.dt.float32
BF16 = mybir.dt.bfloat16

T = 2048
C = 1024
NH_LOC = 8          # heads per core
HD = 64
J = NH_LOC * HD     # 512 local q/k/v channels
N_CORES = 8
QC = 4              # q chunks of 512
TOK_TILES = 16      # token tiles of 128
C_TILES = 8         # contraction tiles of 128 over C
PAIRS = 4           # head pairs per core


def build_nc(debug_taps=False):
    nc = bacc.Bacc("TRN2", target_bir_lowering=False, debug=False)
    dbg = {}
    if debug_taps:
        dbg["y"] = nc.dram_tensor("dbg_y", [PAIRS * 128, T], BF16,
                                  kind="ExternalOutput")
        dbg["qt"] = nc.dram_tensor("dbg_qt", [PAIRS * 128, T], BF16,
                                   kind="ExternalOutput")
        dbg["kt"] = nc.dram_tensor("dbg_kt", [PAIRS * 128, T], BF16,
                                   kind="ExternalOutput")
        dbg["v"] = nc.dram_tensor("dbg_v", [128, TOK_TILES * NH_LOC * 65],
                                  BF16, kind="ExternalOutput")

    # Weights arrive host-rearranged partition-major so every weight DMA is
    # 128 fat contiguous descriptors (the SWDGE ring holds only 1024 and
    # HWDGE is a serial ~665ns/instruction resource).
    x_d = nc.dram_tensor("x", [T, C], BF16, kind="ExternalInput")
    wq_d = nc.dram_tensor("wq", [128, 4, C_TILES, 128], BF16,
                          kind="ExternalInput")
    wk_d = nc.dram_tensor("wk", [128, 4, C_TILES, 128], BF16,
                          kind="ExternalInput")
    wv_d = nc.dram_tensor("wv", [128, C_TILES, J], BF16, kind="ExternalInput")
    bqk_d = nc.dram_tensor("bqk", [128, 8], F32, kind="ExternalInput")
    bv_d = nc.dram_tensor("bv", [J], BF16, kind="ExternalInput")
    wp_d = nc.dram_tensor("wp", [128, PAIRS, C], BF16, kind="ExternalInput")
    out_d = nc.dram_tensor("out", [T, C], F32, kind="ExternalOutput")

    with tile.TileContext(nc) as tc, ExitStack() as ctx:
        const = ctx.enter_context(tc.tile_pool(name="const", bufs=1))
        wpool = ctx.enter_context(tc.tile_pool(name="w", bufs=1))
        qkv = ctx.enter_context(tc.tile_pool(name="qkv", bufs=1))
        ypool = ctx.enter_context(tc.tile_pool(name="y", bufs=1))
        wk = ctx.enter_context(tc.tile_pool(name="wk", bufs=1))

        # ---- resident weights (bf16) and x^T ----
        bqk_sb = const.tile([128, 8], F32)
        bv_sb = const.tile([1, J], BF16)
        wv_sb = wpool.tile([128, C_TILES, J], BF16, name="wv")
        wq_sb = wpool.tile([128, 4, C_TILES, 128], BF16, name="wq")
        wk_sb = wpool.tile([128, 4, C_TILES, 128], BF16, name="wk")
        wp_sb = wpool.tile([128, PAIRS, C], BF16, name="wp")
        xT = qkv.tile([128, C_TILES, T], BF16, name="xT")

        # DMA order is startup-critical. Each HWDGE queue only sustains ~2
        # DMAs in flight, so use both queues (SP carries x^T transposes,
        # Activation carries weights) with big transfers: the XBAR maps
        # in [tok, c] -> out[p, ct, t] with c = ct*128 + p directly.
        nc.scalar.dma_start(wv_sb[:, 0:4, :], wv_d[:, 0:4, :])
        nc.sync.dma_start(xT[:, 0:4, 0:512], x_d[0:512, 0:512],
                          transpose=True)
        nc.scalar.dma_start(wv_sb[:, 4:C_TILES, :], wv_d[:, 4:C_TILES, :])
        nc.sync.dma_start(xT[:, 4:8, 0:512], x_d[0:512, 512:1024],
                          transpose=True)
        nc.scalar.dma_start(wq_sb[:, 0:2], wq_d[:, 0:2])
        nc.scalar.dma_start(wq_sb[:, 2:4], wq_d[:, 2:4])
        nc.scalar.dma_start(bqk_sb, bqk_d[:, :])
        nc.scalar.dma_start(bv_sb, bv_d[:].rearrange("(a n) -> a n", a=1))
        nc.sync.dma_start(xT[:, :, 512:1024], x_d[512:1024, :],
                          transpose=True)
        # wk rides the sync queue behind chunk-1 x^T so the (cheap) wp and
        # chunk 2-3 x^T transfers can never jump ahead of it on the DMA bus
        nc.sync.dma_start(wk_sb[:, 0:2], wk_d[:, 0:2])
        nc.sync.dma_start(wk_sb[:, 2:4], wk_d[:, 2:4])
        nc.scalar.dma_start(wp_sb, wp_d[:, :, :])
        nc.sync.dma_start(xT[:, :, 1024:1536], x_d[1024:1536, :],
                          transpose=True)
        nc.sync.dma_start(xT[:, :, 1536:T], x_d[1536:T, :],
                          transpose=True)

        # ---- constants ----
        # triu2[p, c, f] = 1 iff f >= p, duplicated over c: masks the causal
        # diagonal 128-block of both heads' P in one tensor_tensor op.
        triu2 = const.tile([128, 2, 128], BF16)
        nc.gpsimd.memset(triu2, 0.0)
        nc.gpsimd.affine_select(
            out=triu2, in_=triu2, compare_op=mybir.AluOpType.is_gt,
            fill=1.0, base=0, pattern=[[0, 2], [-1, 128]],
            channel_multiplier=1)
        ones_row = const.tile([1, 128], BF16)
        nc.vector.memset(ones_row, 1.0)
        # bf16 identity: folds the last chunk's stashed partial projection
        # back into PSUM via the PE (no DVE adds on the critical tail)
        ident_bf = const.tile([128, 128], BF16)
        make_identity(nc, ident_bf)
        # selab[p, f] = 1 iff f in [64p, 64p+64): head selector for the
        # reciprocal broadcast matmul (partition-1 memsets are illegal).
        selab = const.tile([2, 128], F32)
        nc.gpsimd.memset(selab, 1.0)
        nc.gpsimd.affine_select(
            out=selab, in_=selab, compare_op=mybir.AluOpType.is_ge,
            fill=0.0, base=0, pattern=[[1, 128]], channel_multiplier=-64)
        nc.gpsimd.affine_select(
            out=selab, in_=selab, compare_op=mybir.AluOpType.is_ge,
            fill=0.0, base=63, pattern=[[-1, 128]], channel_multiplier=64)
        selab_r = const.tile([2, 128], mybir.dt.float32r)
        nc.vector.tensor_copy(selab_r, selab)
        # e65[p, f] = 1 iff p == 64: broadcasts the denominator row of the
        # last group's O' accumulator to all 128 partitions via the PE,
        # skipping the cross-partition sums DMA on the exposed tail chain.
        # Unused rows are zeroed so 0 * garbage never makes a NaN. Built
        # with memset + DMA (stride-0 affine_select patterns misbehave on
        # real GpSimd hardware).
        ones_f = const.tile([1, 128], F32)
        nc.vector.memset(ones_f, 1.0)
        e65 = const.tile([128, 128], F32)
        nc.vector.memset(e65, 0.0)
        nc.sync.dma_start(e65[64:65, :], ones_f[0:1, :])
        e65_r = const.tile([128, 128], mybir.dt.float32r)
        nc.vector.tensor_copy(e65_r, e65)
        z65 = const.tile([128, 1024], F32)
        nc.vector.memset(z65, 0.0)
        stgr65 = const.tile([128, 1024], mybir.dt.float32r)
        nc.vector.tensor_copy(stgr65, z65)

        # ---- persistent activations ----
        qt_sb = [qkv.tile([128, T], BF16, name=f"qt{p}") for p in range(PAIRS)]
        kt_sb = [qkv.tile([128, T], BF16, name=f"kt{p}") for p in range(PAIRS)]
        # v layout [128, tt, head, 65]: col 64 = ones (softmax denominator)
        v_sb = qkv.tile([128, TOK_TILES, NH_LOC, 65], BF16, name="v")
        nc.vector.memset(v_sb[:, :, :, 64:65], 1.0)
        y_sb = [ypool.tile([128, T], BF16, name=f"y{p}") for p in range(PAIRS)]

        # ====== fused pipeline: qkv projection chunks overlap attention ====
        # One PSUM pool for the whole kernel (8 banks):
        #   S    [128,1024] x2  exp-pipeline score tiles            4 banks
        #   O    [65,512]   x2  O' accumulators (o_a, o_b)          2 banks
        #   acc  [128,512]  x1  qkv-projection + c_proj accums      1 bank
        #   bcpo [128,512]  x1  recip broadcasts + c_proj           1 bank
        # Chunk 0 runs before attention and borrows the idle S/O banks for
        # 8 parallel ct-incremental accumulators.
        with tc.tile_pool(name="ps", bufs=1, space="PSUM") as psb:
            pending = []

            def flush_one():
                if pending:
                    pending.pop(0)()

            def flush_pending():
                while pending:
                    pending.pop(0)()

            def wqk_view(ct, jt):
                if jt < 4:
                    return wq_sb[:, jt, ct, :]
                return wk_sb[:, jt - 4, ct, :]

            def qk_finish(jt, qc, pm):
                # late chunks run during ScalarE-heavy attention: use DVE
                dst = qt_sb[jt] if jt < 4 else kt_sb[jt - 4]
                if qc >= 2:
                    nc.vector.tensor_scalar_add(
                        dst[:, qc * 512:(qc + 1) * 512], pm,
                        bqk_sb[:, jt:jt + 1])
                else:
                    nc.scalar.activation(
                        dst[:, qc * 512:(qc + 1) * 512], pm,
                        mybir.ActivationFunctionType.Identity,
                        bias=bqk_sb[:, jt:jt + 1])

            def v_finish(tta, pv):
                nc.tensor.matmul(pv, ones_row, bv_sb, start=False, stop=True)
                src = pv.rearrange("p (h w) -> p h w", h=NH_LOC)
                if tta >= 8:
                    nc.vector.tensor_copy(v_sb[:, tta, :, 0:64], src)
                else:
                    nc.scalar.copy(v_sb[:, tta, :, 0:64], src)

            # ---- chunk 0: ct-incremental over 8 PSUM accumulators ----
            a0 = psb.tile([128, 512], F32, tag="acc", bufs=1, name="a0")
            a1 = psb.tile([128, 512], F32, tag="bcpo", bufs=1, name="a1")
            s0 = psb.tile([128, 1024], F32, tag="S", bufs=2, name="s0")
            s1 = psb.tile([128, 1024], F32, tag="S", bufs=2, name="s1")
            o0 = psb.tile([128, 512], F32, tag="O", bufs=2, name="o0")
            o1 = psb.tile([128, 512], F32, tag="O", bufs=2, name="o1")
            accs = [a0, a1, s0[:, 0:512], s0[:, 512:1024],
                    s1[:, 0:512], s1[:, 512:1024], o0, o1]
            # wave 1: V' tt 0..3 ct-incremental (wv lands first)
            for ct in range(C_TILES):
                for tt in range(4):
                    nc.tensor.matmul(
                        accs[tt], xT[:, ct, tt * 128:(tt + 1) * 128],
                        wv_sb[:, ct, :], start=(ct == 0), stop=False)
            for tt in range(4):
                v_finish(tt, accs[tt])
            # wave 2: all Q (wq lands before wk), then all K
            for i, jt in enumerate([0, 1, 2, 3]):
                pm = accs[4 + i]
                for ct in range(C_TILES):
                    nc.tensor.matmul(
                        pm, wqk_view(ct, jt), xT[:, ct, 0:512],
                        start=(ct == 0), stop=(ct == C_TILES - 1))
                qk_finish(jt, 0, pm)
            for i, jt in enumerate([4, 5, 6, 7]):
                pm = accs[i]
                for ct in range(C_TILES):
                    nc.tensor.matmul(
                        pm, wqk_view(ct, jt), xT[:, ct, 0:512],
                        start=(ct == 0), stop=(ct == C_TILES - 1))
                qk_finish(jt, 0, pm)

            # qkv pieces alternate between the acc and bcpo banks so a
            # piece's accumulation never stalls on the previous piece's
            # PSUM->SBUF read (2-deep software pipeline).
            piece_tag = [0]

            def next_tag():
                piece_tag[0] ^= 1
                return "acc" if piece_tag[0] else "bcpo"

            def a_pieces(qc):
                """Emit-later closures computing Q^T/K^T, V' for chunk qc."""
                pieces = []

                def qk_piece(jt):
                    def run():
                        pm = psb.tile([128, 512], F32, tag=next_tag(),
                                      bufs=1, name="pm")
                        for ct in range(C_TILES):
                            nc.tensor.matmul(
                                pm, wqk_view(ct, jt),
                                xT[:, ct, qc * 512:(qc + 1) * 512],
                                start=(ct == 0), stop=(ct == C_TILES - 1))
                        qk_finish(jt, qc, pm)
                    return run

                def v_piece(tt):
                    def run():
                        tta = qc * 4 + tt
                        pv = psb.tile([128, J], F32, tag=next_tag(),
                                      bufs=1, name="pv")
                        for ct in range(C_TILES):
                            nc.tensor.matmul(
                                pv,
                                xT[:, ct, tta * 128:(tta + 1) * 128],
                                wv_sb[:, ct, :],
                                start=(ct == 0), stop=False)
                        v_finish(tta, pv)
                    return run

                for jt in range(8):
                    pieces.append(qk_piece(jt))
                for tt in range(4):
                    pieces.append(v_piece(tt))
                return pieces

            def emit_tail(p, q0, o_a, o_b, last=False):
                # head A rows land aligned; stage sums + head B rows
                nc.vector.tensor_copy(y_sb[p][0:64, q0:q0 + 512],
                                      o_a[0:64, :])
                stg_b = wk.tile([64, 512], BF16, tag="stgb", bufs=3)
                nc.vector.tensor_copy(stg_b, o_b[0:64, :])
                stg_s = wk.tile([65, 1024], F32, tag="stgs", bufs=2)
                nc.vector.tensor_copy(stg_s[64:65, 0:512], o_a[64:65, :])
                nc.vector.tensor_copy(stg_s[64:65, 512:1024], o_b[64:65, :])
                nc.sync.dma_start(y_sb[p][64:128, q0:q0 + 512], stg_b)
                if last:
                    # exposed tail: broadcast the raw denominator row via
                    # the PE, then take the reciprocal on the full tile
                    # (single-partition reciprocal at offset 64 silently
                    # returns zeros on real hardware), skipping the ~3us
                    # cross-partition DMA round trip.
                    nc.vector.tensor_copy(stgr65[64:65, :],
                                          stg_s[64:65, :])

                    def fin():
                        # bc2[p, :] = (sum_a | sum_b) for every p: head A
                        # rows read the left half, head B rows the right
                        bc2 = psb.tile([128, 1024], F32, tag="S", bufs=2,
                                       name="bc2")
                        nc.tensor.matmul(bc2[:, 0:512], e65_r,
                                         stgr65[:, 0:512],
                                         start=True, stop=True)
                        nc.tensor.matmul(bc2[:, 512:1024], e65_r,
                                         stgr65[:, 512:1024],
                                         start=True, stop=True)
                        recs = wk.tile([128, 1024], F32, tag="ob3", bufs=3)
                        nc.vector.reciprocal_approx_fast(recs, bc2)
                        nc.vector.tensor_mul(y_sb[p][0:64, q0:q0 + 512],
                                             y_sb[p][0:64, q0:q0 + 512],
                                             recs[0:64, 0:512])
                        nc.vector.tensor_mul(y_sb[p][64:128, q0:q0 + 512],
                                             y_sb[p][64:128, q0:q0 + 512],
                                             recs[64:128, 512:1024])
                    pending.append(fin)
                    return
                sums = wk.tile([2, 512], F32, tag="sums", bufs=3)
                nc.sync.dma_start(sums[0:1, :], stg_s[64:65, 0:512])
                nc.sync.dma_start(sums[1:2, :], stg_s[64:65, 512:1024])
                rec = wk.tile([2, 512], F32, tag="rec", bufs=3)
                nc.vector.reciprocal_approx_fast(rec, sums)
                # f32r matmul inputs must come from a rounding producer
                rec_r = wk.tile([2, 512], mybir.dt.float32r, tag="recr",
                                bufs=2)
                nc.vector.tensor_copy(rec_r, rec)

                def fin():
                    # f32r runs 1 cyc/row vs fp32's 4 (values are exact 0/1
                    # selector rows times fp32 reciprocals; f32r's reduced
                    # multiply precision is irrelevant here)
                    bc = psb.tile([128, 512], F32, tag="bcpo", bufs=1,
                                  name="bc")
                    nc.tensor.matmul(bc, selab_r, rec_r,
                                     start=True, stop=True)
                    nc.vector.tensor_mul(y_sb[p][:, q0:q0 + 512],
                                         y_sb[p][:, q0:q0 + 512], bc)
                fin.is_norm_fin = True
                pending.append(fin)

            def make_proj_piece(tt, oc, tag="bcpo", bufs=1, on_scalar=False):
                # one output-projection accumulation for token tile tt
                # (needs all 4 pairs' y columns for tt normalized)
                def proj():
                    po = psb.tile([128, 512], F32, tag=tag, bufs=bufs,
                                  name="po")
                    for p in range(PAIRS):
                        nc.tensor.matmul(
                            po,
                            y_sb[p][:, tt * 128:(tt + 1) * 128],
                            wp_sb[:, p, oc * 512:(oc + 1) * 512],
                            start=(p == 0), stop=(p == PAIRS - 1))
                    ob = wk.tile([128, 512], F32, tag="ob", bufs=4)
                    if on_scalar:
                        nc.scalar.copy(ob, po)
                    else:
                        nc.vector.tensor_copy(ob, po)
                    nc.gpsimd.dma_start(
                        out_d[tt * 128:(tt + 1) * 128,
                              oc * 512:(oc + 1) * 512], ob)
                return proj

            proj3_stash = {}

            def make_proj3_part(tt, oc):
                # pairs 0..2 of the last chunk's projection, stashed to SBUF
                # as bf16 so the tail can re-add it through the PE
                def part():
                    po = psb.tile([128, 512], F32, tag=next_tag(), bufs=1,
                                  name="po3p")
                    for p in range(3):
                        nc.tensor.matmul(
                            po,
                            y_sb[p][:, tt * 128:(tt + 1) * 128],
                            wp_sb[:, p, oc * 512:(oc + 1) * 512],
                            start=(p == 0), stop=(p == 2))
                    st = wk.tile([128, 512], BF16, tag="stash", bufs=8)
                    nc.vector.tensor_copy(st, po)
                    proj3_stash[(tt, oc)] = st
                return part

            def make_proj3_fin(tt):
                # pair-3 term + PE-folded stash add, one piece per token
                # tile; copies alternate ScalarE/DVE, stores alternate the
                # two HWDGE queues (each sustains only ~2 DMAs in flight)
                def fin3():
                    po = psb.tile([128, 1024], F32, tag="S", bufs=2,
                                  name="po3f")
                    for oc in range(2):
                        nc.tensor.matmul(
                            po[:, oc * 512:(oc + 1) * 512],
                            ident_bf, proj3_stash[(tt, oc)],
                            start=True, stop=False)
                        nc.tensor.matmul(
                            po[:, oc * 512:(oc + 1) * 512],
                            y_sb[3][:, tt * 128:(tt + 1) * 128],
                            wp_sb[:, 3, oc * 512:(oc + 1) * 512],
                            start=False, stop=True)
                    ob = wk.tile([128, 1024], F32, tag="ob3", bufs=3)
                    if tt % 2:
                        nc.scalar.copy(ob, po)
                        nc.scalar.dma_start(
                            out_d[tt * 128:(tt + 1) * 128, :], ob)
                    else:
                        nc.vector.tensor_copy(ob, po)
                        nc.sync.dma_start(
                            out_d[tt * 128:(tt + 1) * 128, :], ob)
                return fin3

            a_left = [0] * QC  # un-flushed A pieces per chunk

            def count_piece(piece, qc):
                def run():
                    a_left[qc] -= 1
                    piece()
                return run

            for qc in range(QC):
                q0 = qc * 512
                n_kt = 4 * (qc + 1)
                if qc + 1 < QC:
                    pcs = a_pieces(qc + 1)
                    a_left[qc + 1] = len(pcs)
                    pending.extend(count_piece(pc, qc + 1) for pc in pcs)
                # emission barrier: attention for qc depends on chunk qc's
                # Q/K/V writes being *emitted* (Tile tracks deps in trace
                # order); normally a no-op since pieces drain during qc-1.
                while a_left[qc] > 0:
                    flush_one()
                for p in range(PAIRS):
                    o_a = psb.tile([65, 512], F32, tag="O", bufs=2, name="o_a")
                    o_b = psb.tile([65, 512], F32, tag="O", bufs=2, name="o_b")
                    staged = {}

                    def emit_s(kt):
                        off = max(0, kt * 128 - q0)
                        # S for both heads in one 2-bank psum tile so one
                        # ScalarE exp covers both
                        s_ab = psb.tile([128, 1024], F32, tag="S", bufs=2,
                                        name="s_ab")
                        for half in range(2):
                            r0, r1 = half * 64, half * 64 + 64
                            nc.tensor.matmul(
                                s_ab[:, half * 512 + off:half * 512 + 512],
                                kt_sb[p][r0:r1, kt * 128:(kt + 1) * 128],
                                qt_sb[p][r0:r1, q0 + off:q0 + 512],
                                start=True, stop=True)
                        staged[kt] = (s_ab, off)

                    def emit_consume(kt):
                        s_ab, off = staged.pop(kt)
                        p_ab = wk.tile([128, 1024], BF16, tag="P", bufs=6,
                                       name="p_ab")
                        s3 = s_ab.rearrange("p (c w) -> p c w", c=2)
                        p3 = p_ab.rearrange("p (c w) -> p c w", c=2)
                        nc.scalar.activation(
                            p3[:, :, off:512], s3[:, :, off:512],
                            mybir.ActivationFunctionType.Exp, scale=0.125)
                        if kt * 128 >= q0:  # causal diagonal block
                            nc.vector.tensor_mul(
                                p3[:, :, off:off + 128],
                                p3[:, :, off:off + 128], triu2)
                        first, last = (kt == 0), (kt == n_kt - 1)
                        nc.tensor.matmul(o_a[:, off:512],
                                         v_sb[:, kt, 2 * p, :],
                                         p_ab[:, off:512],
                                         start=first, stop=last)
                        nc.tensor.matmul(o_b[:, off:512],
                                         v_sb[:, kt, 2 * p + 1, :],
                                         p_ab[:, 512 + off:1024],
                                         start=first, stop=last)

                    # software pipeline: emit S(kt) one step ahead of its
                    # exp/mask/PV consumers so PE never waits for ScalarE.
                    # Pending PE-only pieces are spread evenly across the
                    # remaining attention windows of this chunk: attention
                    # alone is ScalarE-bound (~1038ns vs ~853ns PE per
                    # k-tile), so the pieces are what keep PE busy.
                    quota = -(-len(pending) // (PAIRS - p))
                    flushed = 0
                    for kt in range(n_kt + 1):
                        if kt < n_kt:
                            emit_s(kt)
                        if kt >= 1:
                            target = quota * kt // n_kt
                            while flushed < target and pending:
                                # hold a normalization finish back ~4 k-tiles
                                # so PE never waits on its sums-DMA round
                                # trip (~3.5us)
                                if (kt < 4 and
                                        getattr(pending[0], "is_norm_fin",
                                                False)):
                                    break
                                flush_one()
                                flushed += 1
                            emit_consume(kt - 1)
                    emit_tail(p, q0, o_a, o_b,
                              last=(qc == QC - 1 and p == PAIRS - 1))
                    if qc == QC - 1 and p == 2:
                        # last chunk: the projection would otherwise be an
                        # un-overlapped tail after pair 3 finishes. Run the
                        # pairs-0..2 partial sums during pair 3's attention,
                        # stash them in SBUF, and only the pair-3 term +
                        # add remains at the end.
                        for tt in range(qc * 4, qc * 4 + 4):
                            for oc in range(2):
                                pending.append(make_proj3_part(tt, oc))
                if qc == QC - 1:
                    for tt in range(qc * 4, qc * 4 + 4):
                        pending.append(make_proj3_fin(tt))
                else:
                    for tt in range(qc * 4, qc * 4 + 4):
                        for oc in range(2):
                            pending.append(make_proj_piece(tt, oc))
            flush_pending()
            if debug_taps:
                for p in range(PAIRS):
                    nc.sync.dma_start(dbg["y"][p * 128:(p + 1) * 128, :],
                                      y_sb[p])
                    nc.sync.dma_start(dbg["qt"][p * 128:(p + 1) * 128, :],
                                      qt_sb[p])
                    nc.sync.dma_start(dbg["kt"][p * 128:(p + 1) * 128, :],
                                      kt_sb[p])
                nc.sync.dma_start(
                    dbg["v"], v_sb.rearrange("p a b c -> p (a b c)"))

    nc.compile()
    return nc


_NC_CACHE = {}


def _get_nc():
    if "nc" not in _NC_CACHE:
        _NC_CACHE["nc"] = build_nc()
    return _NC_CACHE["nc"]


def _pmajor(w):
    """[C_TILES*128, F] -> [128, C_TILES, F] partition-major, contiguous."""
    ct = w.shape[0] // 128
    return np.ascontiguousarray(
        w.reshape(ct, 128, w.shape[1]).transpose(1, 0, 2))


def shard_inputs(x, W_attn, b_attn, W_proj):
    """Per-core input maps. Core c: batch c//2, head group c%2."""
    bf = ml_dtypes.bfloat16
    x = np.asarray(x, dtype=np.float32)
    W_attn = np.asarray(W_attn, dtype=np.float32)
    b_attn = np.asarray(b_attn, dtype=np.float32)
    W_proj = np.asarray(W_proj, dtype=np.float32)
    in_maps = []
    for c in range(N_CORES):
        b, hg = c // 2, c % 2
        qs, ks, vs = hg * J, C + hg * J, 2 * C + hg * J
        # wq/wk jt-major: [p, jt, ct, jcol]
        wq = np.ascontiguousarray(
            W_attn[:, qs:qs + J].astype(bf)
            .reshape(8, 128, 4, 128).transpose(1, 2, 0, 3))
        wkk = np.ascontiguousarray(
            W_attn[:, ks:ks + J].astype(bf)
            .reshape(8, 128, 4, 128).transpose(1, 2, 0, 3))
        wv = _pmajor(W_attn[:, vs:vs + J].astype(bf))
        # bqk[p, jt]: bias for channel jt*128 + p (q for jt<4, k for jt>=4)
        bqk = np.ascontiguousarray(
            np.concatenate([b_attn[qs:qs + J], b_attn[ks:ks + J]])
            .reshape(8, 128).T)
        bv = np.ascontiguousarray(b_attn[vs:vs + J]).astype(bf)
        wp = _pmajor(W_proj[hg * J:(hg + 1) * J, :].astype(bf))
        in_maps.append({
            "x": np.ascontiguousarray(x[b]).astype(bf),
            "wq": wq, "wk": wkk, "wv": wv, "bqk": bqk, "bv": bv, "wp": wp,
        })
    return in_maps


def kernel(x, W_attn, b_attn, W_proj, b_proj):
    nc = _get_nc()
    in_maps = shard_inputs(x, W_attn, b_attn, W_proj)
    res = run_bass_kernel_spmd(nc, in_maps, list(range(N_CORES)))
    b_proj = np.asarray(b_proj, dtype=np.float32)
    outs = []
    for b in range(4):
        partial = res.results[2 * b]["out"] + res.results[2 * b + 1]["out"]
        outs.append(partial + b_proj[None, :])
    return np.stack(outs, axis=0)
